# revision 2
# baseline (speedup 1.0000x reference)
"""Causal self-attention (GPT-style, B=2 S=2048 E=1024 H=16) on 8 trn2 cores.

Sharding: data-parallel over batch (2) x tensor-parallel over heads (4 heads
per core).  Core c handles batch c//4 and heads 4*(c%4) .. +4.  Each core
computes a partial output projection (its 256 head-dims against the matching
W_proj rows); the host sums the 4 partials per batch and adds b_proj.

Per-core kernel layout tricks:
  * Q^T / K^T are produced directly transposed ([d, s]) by using the weight
    slice as the matmul stationary operand, so attention needs no transposes.
  * Scores are computed transposed (S^T[k, q] = K @ Q^T) with two heads packed
    into the 128-row PE array (each head only uses K=64 contraction rows).
  * Softmax skips max-subtraction (scores are bounded for this problem's
    scale); the denominator falls out of an appended ones-column in V.
  * Causal masking = additive -1e30 tiles (host precomputed) on the 4
    diagonal-straddling tiles of each q-chunk.
  * exp(attn)^T tiles feed the PV matmul as the moving operand with V
    stationary; the output arrives transposed ([d, s]) which is exactly the
    stationary operand layout the final projection needs.
"""

import os

import numpy as np

import concourse.bass as bass
import concourse.tile as tile
from concourse import bacc, mybir
from concourse import bass_utils

F32 = mybir.dt.float32
F32R = mybir.dt.float32r

B, S, E, H = 2, 2048, 1024, 16
D = 64            # head dim
NCORES = 8
HPC = 4           # heads per core
DQ = HPC * D      # 256: per-core q/k/v width
KT = 8            # number of 128-row K tiles over E
P = 128
NEG = -1.0e30
SCALE = float(D) ** -0.5

# float32r runs the PE at 4x the fp32 rate (1 cycle/row vs 4) with slightly
# reduced multiply precision.  Flipped on only if hardware accuracy allows.
USE_FP32R = os.environ.get("KERNEL_FP32R", "1") == "1"

_PROGRAM_CACHE = {}


def _mm_dt(ap):
    return ap.bitcast(F32R) if USE_FP32R else ap


def build_program():
    """Build + compile the per-core Tile program (cached per process)."""
    key = USE_FP32R
    if key in _PROGRAM_CACHE:
        return _PROGRAM_CACHE[key]

    nc = bacc.Bacc("TRN2", target_bir_lowering=False, debug=False)

    # All DRAM inputs are host-prepacked into their exact SBUF layouts
    # ([128 partitions, free]) so every load is a plain 2D copy.
    xt = nc.dram_tensor("xt", [P, KT * S], F32, kind="ExternalInput")
    wqk = nc.dram_tensor("wqk", [P, KT * 512], F32, kind="ExternalInput")
    wv = nc.dram_tensor("wv", [P, KT * 260], F32, kind="ExternalInput")
    bqk = nc.dram_tensor("bqk", [P, 4], F32, kind="ExternalInput")
    bv = nc.dram_tensor("bv", [P, 260], F32, kind="ExternalInput")
    wp = nc.dram_tensor("wp", [P, 2 * E], F32, kind="ExternalInput")
    msk = nc.dram_tensor("msk", [P, 4 * 512], F32, kind="ExternalInput")
    y = nc.dram_tensor("y", [S, E], F32, kind="ExternalOutput")

    Exp = mybir.ActivationFunctionType.Exp
    Ident = mybir.ActivationFunctionType.Identity

    with tile.TileContext(nc) as tc:
        with (
            tc.tile_pool(name="consts", bufs=1) as consts,
            tc.tile_pool(name="xin", bufs=1) as xin,
            tc.tile_pool(name="work", bufs=1) as work,
        ):
            wqk_sb = consts.tile([P, KT * 512], F32)
            nc.sync.dma_start(out=wqk_sb[:], in_=wqk[:])
            wv_sb = consts.tile([P, KT * 260], F32)
            nc.sync.dma_start(out=wv_sb[:], in_=wv[:])
            bqk_sb = consts.tile([P, 4], F32)
            nc.sync.dma_start(out=bqk_sb[:], in_=bqk[:])
            bv_sb = consts.tile([P, 260], F32)
            nc.sync.dma_start(out=bv_sb[:], in_=bv[:])
            wp_sb = consts.tile([P, 2 * E], F32)
            nc.sync.dma_start(out=wp_sb[:], in_=wp[:])
            msk_sb = consts.tile([P, 4 * 512], F32)
            nc.sync.dma_start(out=msk_sb[:], in_=msk[:])
            ones_sb = consts.tile([1, 64], F32)
            nc.vector.memset(ones_sb[:], 1.0)

            # Persistent intermediates.
            # qkt: [d, s] for m-blocks (Q01 | Q23 | K01 | K23), 2048 cols each.
            qkt_sb = consts.tile([P, 4 * S], F32)
            # v: 16 s-blocks of [128, 4 heads * 65] (65th col becomes ones).
            v_sb = consts.tile([P, 16 * 260], F32)
            # outT: [d, s] per head-pair tile (rows 0:64 head even, 64:128 odd).
            out_sb = consts.tile([P, 2 * S], F32)

            # ---- Phase 1+2: QKV projection (streams x^T column slabs) ----
            with tc.tile_pool(name="ps23", space="PSUM", bufs=1) as ps23:
                for nch in range(4):
                    xsl = []
                    for k in range(KT):
                        t = xin.tile(
                            [P, 512], F32, name=f"xsl{k}", tag=f"xsl{k}", bufs=2
                        )
                        nc.sync.dma_start(
                            out=t[:],
                            in_=xt[:, k * S + nch * 512 : k * S + nch * 512 + 512],
                        )
                        xsl.append(t)
                    # Q^T / K^T: weights stationary -> output lands [d, s].
                    for m in range(4):
                        ps = ps23.tile([P, 512], F32, name="ps_qkt",
                                       tag="ps_qkt", bufs=2)
                        for k in range(KT):
                            nc.tensor.matmul(
                                ps[:],
                                _mm_dt(wqk_sb[:, k * 512 + m * P : k * 512 + (m + 1) * P]),
                                _mm_dt(xsl[k][:]),
                                start=(k == 0),
                                stop=(k == KT - 1),
                            )
                        nc.scalar.activation(
                            qkt_sb[:, m * S + nch * 512 : m * S + nch * 512 + 512],
                            ps[:],
                            Ident,
                            bias=bqk_sb[:, m : m + 1],
                            scale=1.0,
                        )
                    # V (+ ones column): x^T slices stationary -> [s, d] layout.
                    for j in range(4):
                        sb_idx = nch * 4 + j
                        psv = ps23.tile([P, 260], F32, name="ps_v",
                                        tag="ps_v", bufs=2)
                        for k in range(KT):
                            nc.tensor.matmul(
                                psv[:],
                                _mm_dt(xsl[k][:, j * P : (j + 1) * P]),
                                _mm_dt(wv_sb[:, k * 260 : (k + 1) * 260]),
                                start=(k == 0),
                                stop=(k == KT - 1),
                            )
                        nc.vector.tensor_add(
                            v_sb[:, sb_idx * 260 : (sb_idx + 1) * 260],
                            psv[:],
                            bv_sb[:],
                        )

            # ---- Phase 3: attention, two heads at a time (PE row packing) ----
            with tc.tile_pool(name="ps4", space="PSUM", bufs=1) as ps4:
                for hp in range(2):
                    qcol = hp * S          # Q m-block column base in qkt_sb
                    kcol = (2 + hp) * S    # K m-block column base
                    for qc in range(4):
                        kmax = 4 * qc + 4
                        oA = ps4.tile([65, 512], F32, name="oA", tag="oA", bufs=1)
                        oB = ps4.tile([65, 512], F32, name="oB", tag="oB", bufs=1)
                        pending = None
                        for kb in range(kmax):
                            sA = ps4.tile([P, 512], F32, name="sA", tag="sA", bufs=2)
                            sB = ps4.tile([P, 512], F32, name="sB", tag="sB", bufs=2)
                            nc.tensor.matmul(
                                sA[:],
                                _mm_dt(qkt_sb[0:64, kcol + kb * P : kcol + (kb + 1) * P]),
                                _mm_dt(qkt_sb[0:64, qcol + qc * 512 : qcol + qc * 512 + 512]),
                                start=True,
                                stop=True,
                            )
                            nc.tensor.matmul(
                                sB[:],
                                _mm_dt(qkt_sb[64:128, kcol + kb * P : kcol + (kb + 1) * P]),
                                _mm_dt(qkt_sb[64:128, qcol + qc * 512 : qcol + qc * 512 + 512]),
                                start=True,
                                stop=True,
                            )
                            j = kb - 4 * qc
                            if j >= 0:  # diagonal-straddling tile: causal mask
                                nc.vector.tensor_add(
                                    sA[:], sA[:], msk_sb[:, j * 512 : (j + 1) * 512]
                                )
                                nc.vector.tensor_add(
                                    sB[:], sB[:], msk_sb[:, j * 512 : (j + 1) * 512]
                                )
                            eA = work.tile([P, 512], F32, name="eA", tag="eA", bufs=3)
                            eB = work.tile([P, 512], F32, name="eB", tag="eB", bufs=3)
                            nc.scalar.activation(eA[:], sA[:], Exp, scale=SCALE)
                            nc.scalar.activation(eB[:], sB[:], Exp, scale=SCALE)
                            # one-iteration software pipeline: PV matmul for the
                            # previous kb issues after this kb's scores, so the
                            # PE never waits on the ACT exp.
                            if pending is not None:
                                pkb, peA, peB = pending
                                _pv(nc, oA, oB, v_sb, hp, pkb, peA, peB, kmax)
                            pending = (kb, eA, eB)
                        pkb, peA, peB = pending
                        _pv(nc, oA, oB, v_sb, hp, pkb, peA, peB, kmax)

                        # normalize: recip of ones-column, PE-broadcast it
                        # across partitions, multiply, store into out_sb.
                        rA = work.tile([1, 512], F32, name="rA", tag="rA", bufs=2)
                        rB = work.tile([1, 512], F32, name="rB", tag="rB", bufs=2)
                        nc.vector.reciprocal(rA[:], oA[64:65, :])
                        nc.vector.reciprocal(rB[:], oB[64:65, :])
                        bcA = ps4.tile([64, 512], F32, name="bcA", tag="bcA", bufs=1)
                        bcB = ps4.tile([64, 512], F32, name="bcB", tag="bcB", bufs=1)
                        nc.tensor.matmul(bcA[:], _mm_dt(ones_sb[:]), _mm_dt(rA[:]),
                                         start=True, stop=True)
                        nc.tensor.matmul(bcB[:], _mm_dt(ones_sb[:]), _mm_dt(rB[:]),
                                         start=True, stop=True)
                        sbA = work.tile([64, 512], F32, name="sbA", tag="sbA", bufs=2)
                        sbB = work.tile([64, 512], F32, name="sbB", tag="sbB", bufs=2)
                        nc.scalar.copy(sbA[:], bcA[:])
                        nc.scalar.copy(sbB[:], bcB[:])
                        nc.vector.tensor_mul(
                            out_sb[0:64, hp * S + qc * 512 : hp * S + qc * 512 + 512],
                            oA[0:64, :],
                            sbA[:],
                        )
                        nc.vector.tensor_mul(
                            out_sb[64:128, hp * S + qc * 512 : hp * S + qc * 512 + 512],
                            oB[0:64, :],
                            sbB[:],
                        )

            # ---- Phase 4: output projection (partial y over this core's d) ----
            with tc.tile_pool(name="ps5", space="PSUM", bufs=1) as ps5:
                for sb in range(16):
                    ysb = work.tile([P, E], F32, name="ysb", tag="ysb", bufs=3)
                    for ec in range(2):
                        py = ps5.tile([P, 512], F32, name="py", tag="py", bufs=2)
                        for t in range(2):
                            nc.tensor.matmul(
                                py[:],
                                _mm_dt(out_sb[:, t * S + sb * P : t * S + (sb + 1) * P]),
                                _mm_dt(wp_sb[:, t * E + ec * 512 : t * E + ec * 512 + 512]),
                                start=(t == 0),
                                stop=(t == 1),
                            )
                        nc.vector.tensor_copy(ysb[:, ec * 512 : (ec + 1) * 512], py[:])
                    nc.sync.dma_start(
                        out=y[sb * P : (sb + 1) * P, :], in_=ysb[:]
                    )

    nc.compile()
    _PROGRAM_CACHE[key] = nc
    return nc


def _pv(nc, oA, oB, v_sb, hp, kb, eA, eB, kmax):
    """PV matmuls for one (kb, head-pair): V slice stationary, exp moving."""
    nc.tensor.matmul(
        oA[:],
        _mm_dt(v_sb[:, kb * 260 + (2 * hp) * 65 : kb * 260 + (2 * hp) * 65 + 65]),
        _mm_dt(eA[:]),
        start=(kb == 0),
        stop=(kb == kmax - 1),
    )
    nc.tensor.matmul(
        oB[:],
        _mm_dt(v_sb[:, kb * 260 + (2 * hp + 1) * 65 : kb * 260 + (2 * hp + 1) * 65 + 65]),
        _mm_dt(eB[:]),
        start=(kb == 0),
        stop=(kb == kmax - 1),
    )


def _to_sbuf_layout(a, cols):
    """[KT*128, cols] -> [128, KT*cols] with col block k = K-tile k."""
    return (
        np.ascontiguousarray(
            a.reshape(KT, P, cols).transpose(1, 0, 2).reshape(P, KT * cols)
        )
    )


def _pack_all(x, W_attn, b_attn, W_proj):
    f32 = np.float32
    maps = []
    for core in range(NCORES):
        b, hs = core // 4, (core % 4) * HPC
        m = {}
        xt = np.ascontiguousarray(x[b].T.astype(f32))
        m["xt"] = _to_sbuf_layout(xt, S)
        wq = W_attn[:, hs * D : hs * D + DQ]
        wk = W_attn[:, E + hs * D : E + hs * D + DQ]
        m["wqk"] = _to_sbuf_layout(
            np.concatenate([wq, wk], axis=1).astype(f32), 512
        )
        wv_heads = W_attn[:, 2 * E + hs * D : 2 * E + hs * D + DQ].reshape(
            E, HPC, D
        )
        wva = np.zeros((E, HPC, 65), f32)
        wva[:, :, :D] = wv_heads
        m["wv"] = _to_sbuf_layout(wva.reshape(E, 260), 260)
        m["bqk"] = np.stack(
            [
                b_attn[hs * D : hs * D + P],
                b_attn[hs * D + P : hs * D + DQ],
                b_attn[E + hs * D : E + hs * D + P],
                b_attn[E + hs * D + P : E + hs * D + DQ],
            ],
            axis=1,
        ).astype(f32)
        bv_row = np.zeros((HPC, 65), f32)
        bv_row[:, :D] = b_attn[2 * E + hs * D : 2 * E + hs * D + DQ].reshape(
            HPC, D
        )
        bv_row[:, D] = 1.0
        m["bv"] = np.ascontiguousarray(
            np.broadcast_to(bv_row.reshape(1, 260), (P, 260))
        )
        m["wp"] = np.ascontiguousarray(
            W_proj[hs * D : hs * D + DQ, :]
            .astype(f32)
            .reshape(2, P, E)
            .transpose(1, 0, 2)
            .reshape(P, 2 * E)
        )
        pgrid = np.arange(P)[:, None]
        fgrid = np.arange(512)[None, :]
        m["msk"] = np.concatenate(
            [
                np.where(pgrid + j * P <= fgrid, 0.0, NEG).astype(f32)
                for j in range(4)
            ],
            axis=1,
        )
        maps.append(m)
    return maps


LAST_RESULTS = None


def kernel(x, W_attn, b_attn, W_proj, b_proj):
    global LAST_RESULTS
    x = np.asarray(x, dtype=np.float32)
    W_attn = np.asarray(W_attn, dtype=np.float32)
    b_attn = np.asarray(b_attn, dtype=np.float32)
    W_proj = np.asarray(W_proj, dtype=np.float32)
    b_proj = np.asarray(b_proj, dtype=np.float32)

    nc = build_program()
    in_maps = _pack_all(x, W_attn, b_attn, W_proj)
    res = bass_utils.run_bass_kernel_spmd(nc, in_maps, list(range(NCORES)))
    LAST_RESULTS = res

    y = np.zeros((B, S, E), np.float32)
    for b in range(B):
        acc = res.results[4 * b]["y"].astype(np.float32)
        for i in range(1, 4):
            acc = acc + res.results[4 * b + i]["y"]
        y[b] = acc + b_proj[None, :]
    return y
